# revision 11
# baseline (speedup 1.0000x reference)
"""Trainium2 Bass kernel for DiffusionCoordinateInitializer.

Math: target = latent @ W + b            ([B*N, 1024] @ [1024, 3])
      scan:  x <- a*x + (1-a)*target  over alphas = (steps..1)/steps, x0 = noise
Closed form: x_final = P*noise + (1-P)*target,  P = prod(t/steps) = steps!/steps^steps.

Strategy (pure data parallel over the 32768 rows, 4096 rows/core on 8 cores):
  - Stream latent row-tiles [128, 1024] to SBUF (natural layout, full-BW DMA).
  - TensorE fp32 transpose of each 128x128 block into PSUM; the PSUM->SBUF
    copy (DVE/ACT alternating) simultaneously rounds to float32r.
  - Skinny accumulating float32r matmul with the 128x3 W-block stationary
    produces target^T [3, 512] per row-group in PSUM (f32r streams at
    1 cyc/row vs fp32's 4).
  - P*noise and (1-P)*b are folded into the same PSUM accumulation group as
    one rank-4 matmul: lhsT = [[P*I3],[(1-P)*b]], rhs = [[noise^T],[ones]].
  - Result is produced transposed ([3, rows]); host transposes the small
    [32768, 3] output back.
"""

import sys

if "/opt/trn_rl_repo" not in sys.path:
    sys.path.insert(0, "/opt/trn_rl_repo")

from contextlib import ExitStack

import numpy as np

import concourse.bacc as bacc
import concourse.bass as bass
import concourse.mybir as mybir
import concourse.tile as tile
from concourse.bass_utils import run_bass_kernel_spmd
from concourse.masks import make_identity

F32 = mybir.dt.float32
F32R = mybir.dt.float32r

NCORES = 8
B, N, D, K = 4, 8192, 1024, 3
R_TOTAL = B * N           # 32768 rows
R_CORE = R_TOTAL // NCORES  # 4096 rows per core
RG = 512                  # rows per group (= one PSUM bank of f32)
NG = R_CORE // RG         # 8 row groups per core
RT = RG // 128            # 4 row subtiles of 128 per group
DJ = D // 128             # 8 d-blocks of 128

_BUILT = None


def _build():
    global _BUILT
    if _BUILT is not None:
        return _BUILT

    nc = bacc.Bacc(
        "TRN2", debug=False, target_bir_lowering=False, num_devices=NCORES
    )

    lat = nc.dram_tensor("lat", [NG, RT, 128, D], F32, kind="ExternalInput").ap()
    wb = nc.dram_tensor("wb", [128, DJ * K], F32, kind="ExternalInput").ap()
    s4 = nc.dram_tensor("s4", [K + 1, K], F32, kind="ExternalInput").ap()
    cs4 = nc.dram_tensor("cs4", [K + 1, 1], F32, kind="ExternalInput").ap()
    nz4 = nc.dram_tensor("nz4", [K + 1, R_CORE], F32, kind="ExternalInput").ap()
    ct = nc.dram_tensor("ct", [128, 1], F32, kind="ExternalInput").ap()
    outT = nc.dram_tensor("outT", [K, R_CORE], F32, kind="ExternalOutput").ap()

    with tile.TileContext(nc) as tc, ExitStack() as ctx:
        consts = ctx.enter_context(tc.tile_pool(name="consts", bufs=1))
        latp = ctx.enter_context(tc.tile_pool(name="latp", bufs=6))
        latTp = ctx.enter_context(tc.tile_pool(name="latTp", bufs=10))
        psTp = ctx.enter_context(tc.tile_pool(name="psT", bufs=6, space="PSUM"))
        psOp = ctx.enter_context(tc.tile_pool(name="psO", bufs=2, space="PSUM"))

        ident = consts.tile([128, 128], F32)
        make_identity(nc, ident[:])

        ct_sb = consts.tile([128, 1], F32)
        nc.scalar.dma_start(out=ct_sb[:], in_=ct)

        # W blocks scaled by (1-P), rounded to f32r
        wb_raw = consts.tile([128, DJ * K], F32)
        nc.scalar.dma_start(out=wb_raw[:], in_=wb)
        wb_s = consts.tile([128, DJ * K], F32)
        nc.vector.tensor_scalar_mul(wb_s[:], wb_raw[:], ct_sb[:])
        wb_r = consts.tile([128, DJ * K], F32R)
        nc.vector.tensor_copy(out=wb_r[:], in_=wb_s[:])

        # [[I3],[b]] * [[P],[P],[P],[1-P]] -> [[P*I3],[(1-P)*b]], rounded
        cs4_sb = consts.tile([K + 1, 1], F32)
        nc.scalar.dma_start(out=cs4_sb[:], in_=cs4)
        s4_raw = consts.tile([K + 1, K], F32)
        nc.scalar.dma_start(out=s4_raw[:], in_=s4)
        s4_s = consts.tile([K + 1, K], F32)
        nc.vector.tensor_scalar_mul(s4_s[:], s4_raw[:], cs4_sb[:])
        s4_r = consts.tile([K + 1, K], F32R)
        nc.vector.tensor_copy(out=s4_r[:], in_=s4_s[:])

        # [[noise^T],[ones]] rounded to f32r
        nz4_sb = consts.tile([K + 1, R_CORE], F32)
        nc.scalar.dma_start(out=nz4_sb[:], in_=nz4)
        nz4_r = consts.tile([K + 1, R_CORE], F32R)
        nc.vector.tensor_copy(out=nz4_r[:], in_=nz4_sb[:])

        outT_sb = consts.tile([K, R_CORE], F32)

        for g in range(NG):
            if g == 0:
                # fine-grained first group: transposes can start after 512 KB
                lat_rt = []
                for rt in range(RT):
                    t = latp.tile([128, D], F32, tag="lat0")
                    nc.sync.dma_start(out=t[:], in_=lat[g, rt])
                    lat_rt.append(t)
                lat_slice = lambda rt, j: lat_rt[rt][:, bass.ts(j, 128)]
            else:
                # one big 2 MiB DMA; alternate HWDGE(sync) / SWDGE(gpsimd)
                # rings so per-DMA fixed costs overlap across rings
                lat_g = latp.tile([128, RT, D], F32, tag="latg")
                nc.sync.dma_start(out=lat_g[:], in_=lat[g].rearrange("t p d -> p t d"))
                lat_slice = lambda rt, j: lat_g[:, rt, bass.ts(j, 128)]

            # all transposes + rounding copies for this group first ...
            latTs = []
            for j in range(DJ):
                psT = psTp.tile([128, RG], F32)
                for rt in range(RT):
                    nc.tensor.transpose(
                        psT[:, bass.ts(rt, 128)],
                        lat_slice(rt, j),
                        ident[:],
                    )
                latT = latTp.tile([128, RG], F32R)
                if j % 8 < 5:
                    nc.vector.tensor_copy(out=latT[:], in_=psT[:])
                else:
                    nc.scalar.copy(latT[:], psT[:])
                latTs.append(latT)

            # ... then the dense accumulating matmul burst (no PE stalls)
            psO = psOp.tile([K, RG], F32)
            for j in range(DJ):
                nc.tensor.matmul(
                    psO[:],
                    wb_r[:, bass.ts(j, K)],
                    latTs[j][:],
                    start=(j == 0),
                    stop=False,
                )
            nc.tensor.matmul(
                psO[:], s4_r[:], nz4_r[:, bass.ts(g, RG)], start=False, stop=True
            )
            nc.scalar.copy(outT_sb[:, bass.ts(g, RG)], psO[:])
            nc.scalar.dma_start(
                out=outT[:, g * RG : (g + 1) * RG], in_=outT_sb[:, bass.ts(g, RG)]
            )

    nc.compile()
    _BUILT = nc
    return nc


def _prep_inputs(latent, W, b, noise, steps):
    steps_i = int(steps)
    P = float(np.prod(np.arange(1, steps_i + 1, dtype=np.float64) / steps_i))
    one_minus_P = np.float32(1.0 - P)

    lat_all = np.ascontiguousarray(
        np.asarray(latent, np.float32).reshape(NCORES, NG, RT, 128, D)
    )
    noise_rows = np.asarray(noise, np.float32).reshape(R_TOTAL, K)
    wb = np.ascontiguousarray(
        np.asarray(W, np.float32).reshape(DJ, 128, K).transpose(1, 0, 2).reshape(128, DJ * K)
    )
    s4 = np.concatenate(
        [np.eye(K, dtype=np.float32), np.asarray(b, np.float32).reshape(1, K)], axis=0
    )
    cs4 = np.array([[P]] * K + [[one_minus_P]], dtype=np.float32)
    ct = np.full((128, 1), one_minus_P, np.float32)

    in_maps = []
    for c in range(NCORES):
        nzT = noise_rows[c * R_CORE : (c + 1) * R_CORE].T  # [3, 4096]
        nz4 = np.ascontiguousarray(
            np.concatenate([nzT, np.ones((1, R_CORE), np.float32)], axis=0)
        )
        in_maps.append(
            {
                "lat": lat_all[c],
                "wb": wb,
                "s4": s4,
                "cs4": cs4,
                "nz4": nz4,
                "ct": ct,
            }
        )
    return in_maps


def run(latent, W, b, noise, steps, trace=False, tmpdir=None):
    """Returns (output [4,8192,3], BassKernelResults)."""
    nc = _build()
    in_maps = _prep_inputs(latent, W, b, noise, steps)
    res = run_bass_kernel_spmd(
        nc, in_maps, core_ids=list(range(NCORES)), trace=trace, tmpdir=tmpdir
    )
    outT = np.concatenate(
        [res.results[c]["outT"].T for c in range(NCORES)], axis=0
    )  # [32768, 3]
    return outT.reshape(B, N, K), res


def kernel(latent, W, b, noise, steps):
    out, _ = run(latent, W, b, noise, steps)
    return out
